# revision 1
# baseline (speedup 1.0000x reference)
"""DilatedMDTA Trainium2 kernel: batch-parallel over 8 NeuronCores.

Reference computation (per batch image, c=192 channels, 128x128 pixels):
  qkv  = w_qkv @ x                      (1x1 conv, 192 -> 576)
  qkv  = depthwise 3x3 dilation-2 conv  (per-channel, padding 2)
  q,k,v = split(qkv); per head (8 heads, 24 ch):
  q,k L2-normalized over pixels; attn = softmax(q @ k^T * temp); out = attn @ v
  y = w_proj @ out                      (1x1 conv, 192 -> 192)

Layout strategy (per core, one image):
  - channels on SBUF partitions, pixels on the free dim
  - QKV projection: bf16 matmuls, host-permuted weights pad v-heads to
    32-aligned partition blocks (640 output rows)
  - depthwise conv: 9 PSUM-accumulating diagonal matmuls; taps are pure
    access-pattern offsets on the free (pixel) dim
  - gram (q@kT): PE-transpose q,k tiles to [pixel, ch], per-head matmuls
    col-tiled into 32-aligned PSUM partition blocks, accumulated over all
    128-pixel blocks
  - norms/softmax: fp32 on DVE/ACT; attn @ v: K=32 row+col tiled matmuls
"""

import os
import sys

sys.path.insert(0, "/opt/trn_rl_repo")

import numpy as np
import ml_dtypes

import concourse.bass as bass
import concourse.mybir as mybir
import concourse.tile as tile
from concourse.bass_utils import run_bass_kernel_spmd
from concourse.masks import make_identity

BF16 = mybir.dt.bfloat16
F32 = mybir.dt.float32
AF = mybir.ActivationFunctionType
ALU = mybir.AluOpType
NPBF16 = ml_dtypes.bfloat16

C = 192          # input/output channels
C3 = 576         # qkv channels
CP = 640         # padded qkv channels (v heads 32-aligned)
HEADS = 8
CH = 24          # channels per head
H = 128
W = 128
HW = H * W
NB = 8           # row-bands for phase 1
BROWS = 16       # output rows per band
RB = BROWS + 4   # band buffer rows (2-row halo each side)
EPS = 1e-6

# taps: (dy, dx), identity tap first so it initializes the PSUM chunk
TAPS = [(0, 0), (-2, -2), (-2, 0), (-2, 2), (0, -2), (0, 2), (2, -2), (2, 0), (2, 2)]
# tap index in the 3x3 kernel (ky*3+kx) for each entry of TAPS
TAP_KIDX = [((dy // 2 + 1) * 3 + (dx // 2 + 1)) for dy, dx in TAPS]


def _new_row(o):
    """qkv output channel -> padded row index (v heads 32-aligned)."""
    if o < 384:  # q, k unchanged
        return o
    vc = o - 384
    h, i = divmod(vc, CH)
    return 384 + 32 * h + i


def _prep_host(w_qkv, w_dw, w_proj, temperature):
    """All weight-derived constants, computed on host in numpy."""
    w_qkv = np.asarray(w_qkv, np.float32)
    w_dw = np.asarray(w_dw, np.float32).reshape(C3, 9)
    w_proj = np.asarray(w_proj, np.float32)
    temperature = np.asarray(temperature, np.float32).reshape(HEADS)

    w1p = np.zeros((CP, C), np.float32)
    dwp = np.zeros((CP, 9), np.float32)
    for o in range(C3):
        r = _new_row(o)
        w1p[r] = w_qkv[o]
        dwp[r] = w_dw[o]
    w1t = np.ascontiguousarray(w1p.T).astype(NPBF16)          # [192, 640]

    diag = np.zeros((5, 9, 128, 128), np.float32)
    for t in range(5):
        for n in range(9):
            np.fill_diagonal(diag[t, n], dwp[128 * t:128 * (t + 1), TAP_KIDX[n]])
    diag = diag.astype(NPBF16)                                 # [5, 9, 128, 128]

    w3p = np.zeros((256, C), np.float32)
    for h in range(HEADS):
        for i in range(CH):
            w3p[32 * h + i] = w_proj[:, CH * h + i]
    w3p = w3p.astype(NPBF16)                                   # [256, 192]

    tempg = np.zeros((256, 1), np.float32)
    for h in range(HEADS):
        tempg[32 * h:32 * h + CH, 0] = temperature[h]

    return {"w1t": w1t, "diagw": diag, "w3": w3p, "tempg": tempg}


def _qk_chan_runs(c):
    """qkv channel c (q:0-191, k:192-383) -> (partition, tile-col) runs of 24."""
    runs = []
    start = c
    end = c + CH
    while start < end:
        col = start // 128
        p0 = start % 128
        ln = min(end - start, 128 - p0)
        runs.append((p0, col, start - c, ln))
        start += ln
    return runs


def build_program():
    from concourse import bacc
    nc = bacc.Bacc(None, target_bir_lowering=False)

    x_d = nc.dram_tensor("x", [C, HW], F32, kind="ExternalInput")
    w1t_d = nc.dram_tensor("w1t", [C, CP], BF16, kind="ExternalInput")
    diag_d = nc.dram_tensor("diagw", [5, 9, 128, 128], BF16, kind="ExternalInput")
    w3_d = nc.dram_tensor("w3", [256, C], BF16, kind="ExternalInput")
    tempg_d = nc.dram_tensor("tempg", [256, 1], F32, kind="ExternalInput")
    y_d = nc.dram_tensor("y", [C, HW], F32, kind="ExternalOutput")

    with tile.TileContext(nc) as tc:
        _emit(nc, tc, x_d, w1t_d, diag_d, w3_d, tempg_d, y_d)
    nc.compile()
    return nc


def _emit(nc, tc, x_d, w1t_d, diag_d, w3_d, tempg_d, y_d):
    from contextlib import ExitStack

    ctx = ExitStack()
    with ctx:
        consts = ctx.enter_context(tc.tile_pool(name="consts", bufs=1))
        persist = ctx.enter_context(tc.tile_pool(name="persist", bufs=1))

        # ---- constants ----
        w1a = consts.tile([128, CP], BF16, tag="w1a")
        w1b = consts.tile([64, CP], BF16, tag="w1b")
        diag_sb = consts.tile([128, 5, 9, 128], BF16, tag="diag")
        w3_sb = consts.tile([128, 2, C], BF16, tag="w3")
        ident_bf = consts.tile([128, 128], BF16, tag="idbf")
        ident_f32 = consts.tile([128, 128], F32, tag="idf32")
        tempg_sb = consts.tile([128, 2], F32, tag="tempg")

        nc.sync.dma_start(out=w1a, in_=w1t_d[0:128, :])
        nc.sync.dma_start(out=w1b, in_=w1t_d[128:192, :])
        nc.sync.dma_start(out=diag_sb, in_=diag_d[:].rearrange("t n p c -> p t n c"))
        nc.sync.dma_start(out=w3_sb, in_=w3_d[:].rearrange("(k p) m -> p k m", k=2))
        nc.sync.dma_start(out=tempg_sb, in_=tempg_d[:].rearrange("(s p) o -> p (s o)", s=2))
        make_identity(nc, ident_bf)
        make_identity(nc, ident_f32)

        # ---- persistent SBUF ----
        v_res0 = persist.tile([128, H, W], BF16, tag="vres0")   # v heads 0-3
        v_res1 = persist.tile([128, H, W], BF16, tag="vres1")   # v heads 4-7
        sqacc = persist.tile([128, 3 * NB], F32, tag="sqacc")   # per (tile, band) sums
        attn_sb0 = persist.tile([128, 32], BF16, tag="attn0")
        attn_sb1 = persist.tile([128, 32], BF16, tag="attn1")
        attnT_sb0 = persist.tile([128, 32], BF16, tag="attnT0")
        attnT_sb1 = persist.tile([128, 32], BF16, tag="attnT1")
        attn_sbs = [attn_sb0, attn_sb1]
        attnT_sbs = [attnT_sb0, attnT_sb1]
        v_ress = [v_res0, v_res1]

        nc.vector.memset(attn_sb0, 0.0)
        nc.vector.memset(attn_sb1, 0.0)

        # preload the sqrt ACT table set during phase 1 so the serial
        # norm/softmax section doesn't pay the ~2.7us table load
        warm = persist.tile([128, 1], F32, tag="actwarm")
        nc.vector.memset(warm, 1.0)
        nc.scalar.activation(warm, warm, AF.Sqrt)

        # =========== phase 1: qkv + conv + transposes + gram ===========
        p1 = ExitStack()
        with p1:
            xin = p1.enter_context(tc.tile_pool(name="xin", bufs=4))
            xbf_pool = p1.enter_context(tc.tile_pool(name="xbf", bufs=2))
            qkvb_pool = p1.enter_context(tc.tile_pool(name="qkvb", bufs=2))
            qkb_pool = p1.enter_context(tc.tile_pool(name="qkb", bufs=2))
            qt_pool = p1.enter_context(tc.tile_pool(name="qt", bufs=6))
            qkv_ps = p1.enter_context(tc.tile_pool(name="qkvps", bufs=2, space="PSUM"))
            conv_ps = p1.enter_context(tc.tile_pool(name="convps", bufs=2, space="PSUM"))
            tr_ps = p1.enter_context(tc.tile_pool(name="trps", bufs=2, space="PSUM"))
            g_ps = p1.enter_context(tc.tile_pool(name="gps", bufs=1, space="PSUM"))

            # full-bank pitch (512 f32) so partition-offset slices stay
            # zero-region aligned; only cols 0:CH are used
            g0 = g_ps.tile([128, 512], F32, tag="g0")
            g1 = g_ps.tile([128, 512], F32, tag="g1")
            g_tiles = [g0, g1]
            for gt in g_tiles:
                nc.vector.memset(gt[:, 0:CH], 0.0)

            copy_rr = [0]  # round-robin PSUM->SBUF copies between DVE and ACT

            def psum_copy(dst, src):
                if copy_rr[0] % 2 == 0:
                    nc.vector.tensor_copy(dst, src)
                else:
                    nc.scalar.activation(dst, src, AF.Copy)
                copy_rr[0] += 1

            # qkv_band has 2-col zero pads on each side and zero-padded
            # out-of-image rows so conv taps never clip; double-buffered so
            # the next band's copies overlap this band's conv matmuls
            WP = W + 4
            for b in range(NB):
                buf_base = BROWS * b - 2
                lo = max(0, buf_base)
                hi = min(H, BROWS * b + BROWS + 2)
                l0 = lo - buf_base
                l1 = hi - buf_base

                x_bf = xbf_pool.tile([128, 2, RB, W], BF16, tag="xbf")
                qkv_band = qkvb_pool.tile([128, 5, RB, WP], BF16, tag="qkvband")
                qk_band = qkb_pool.tile([128, 3, BROWS, W], BF16, tag="qkband")
                # zero the column pads (rotating slot may hold stale data)
                nc.vector.memset(qkv_band[:, :, :, 0:2], 0.0)
                nc.vector.memset(qkv_band[:, :, :, WP - 2:WP], 0.0)
                if l0 > 0:
                    nc.vector.memset(qkv_band[:, :, 0:l0, :], 0.0)
                if l1 < RB:
                    nc.vector.memset(qkv_band[:, :, l1:RB, :], 0.0)

                # -- x load + cast --
                for t, pt in ((0, 128), (1, 64)):
                    r = l0
                    while r < l1:
                        rr = min(r + 4, l1)
                        n = rr - r
                        xf = xin.tile([128, 4, W], F32, tag="xf32")
                        nc.sync.dma_start(
                            out=xf[:pt, :n, :],
                            in_=x_d[t * 128:t * 128 + pt,
                                    (buf_base + r) * W:(buf_base + rr) * W]
                            .rearrange("p (r c) -> p r c", c=W),
                        )
                        nc.gpsimd.tensor_copy(x_bf[:pt, t, r:rr, :], xf[:pt, :n, :])
                        r = rr

                # -- qkv projection --
                for mt in range(5):
                    mc = slice(mt * 128, (mt + 1) * 128)
                    r = l0
                    while r < l1:
                        rr = min(r + 4, l1)
                        n = rr - r
                        ps = qkv_ps.tile([128, 4, W], F32, tag="qkvps")
                        nc.tensor.matmul(ps[:, :n, :], w1a[:, mc], x_bf[:, 0, r:rr, :],
                                         start=True, stop=False)
                        nc.tensor.matmul(ps[:, :n, :], w1b[:64, mc], x_bf[:64, 1, r:rr, :],
                                         start=False, stop=True)
                        psum_copy(qkv_band[:, mt, r:rr, 2:2 + W], ps[:, :n, :])
                        r = rr

                # -- depthwise conv: 9 accumulating diag matmuls per chunk --
                for t in range(5):
                    for q in range(BROWS // 4):
                        oy0 = BROWS * b + 4 * q
                        cps = conv_ps.tile([128, 4, W], F32, tag="convps")
                        for ti, (dy, dx) in enumerate(TAPS):
                            r0 = oy0 + dy - buf_base
                            rhs = qkv_band[:, t, r0:r0 + 4, (2 + dx):(2 + dx + W)]
                            nc.tensor.matmul(cps, diag_sb[:, t, ti, :], rhs,
                                             start=(ti == 0), stop=(ti == len(TAPS) - 1))
                        lq = oy0 - BROWS * b
                        if t < 3:
                            psum_copy(qk_band[:, t, lq:lq + 4, :], cps)
                        else:
                            psum_copy(v_ress[t - 3][:, oy0:oy0 + 4, :], cps)

                # -- transpose q,k pixel-blocks + gram --
                for pb in range(BROWS):
                    trp = tr_ps.tile([128, 384], BF16, tag="trps")
                    for t in range(3):
                        nc.tensor.transpose(trp[:, t * 128:(t + 1) * 128],
                                            qk_band[:, t, pb, :], ident_bf)
                    qt = qt_pool.tile([128, 384], BF16, tag="qt")
                    if pb % 2 == 0:
                        nc.vector.tensor_copy(qt, trp)
                    else:
                        nc.scalar.activation(qt, trp, AF.Copy)
                    first = (b == 0 and pb == 0)
                    last = (b == NB - 1 and pb == BROWS - 1)
                    for h in range(HEADS):
                        s, j = divmod(h, 4)
                        nc.tensor.matmul(
                            g_tiles[s][32 * j:32 * j + CH, 0:CH],
                            qt[:, CH * h:CH * h + CH],
                            qt[:, C + CH * h:C + CH * h + CH],
                            start=first, stop=last,
                            tile_position=(0, 32 * j),
                            skip_group_check=True,
                        )

                # -- squared sums for norms (in-place square of qk_band on ACT) --
                for t in range(3):
                    nc.scalar.activation(
                        qk_band[:, t], qk_band[:, t], AF.Square,
                        accum_out=sqacc[:, (t * NB + b):(t * NB + b + 1)],
                    )

            # =========== phase 1.5: norms + softmax + attn^T ===========
            sm = p1.enter_context(tc.tile_pool(name="smax", bufs=1))

            sqsum = sm.tile([128, 3], F32, tag="sqsum")
            nc.vector.tensor_reduce(
                sqsum, sqacc[:].rearrange("p (t b) -> p t b", b=NB),
                axis=mybir.AxisListType.X, op=ALU.add)
            nrm0 = sm.tile([128, 3], F32, tag="nrm0")
            nc.scalar.activation(nrm0, sqsum, AF.Sqrt)
            # one Newton step: nrm = 0.5*(nrm0 + s/nrm0)
            rrec = sm.tile([128, 3], F32, tag="rrec")
            nc.vector.reciprocal(rrec, nrm0)
            t1 = sm.tile([128, 3], F32, tag="t1")
            nc.vector.tensor_mul(t1, sqsum, rrec)
            nc.vector.tensor_add(t1, t1, nrm0)
            rn = sm.tile([128, 3], F32, tag="rn")
            nc.vector.tensor_scalar(out=t1, in0=t1, scalar1=0.5, scalar2=EPS,
                                    op0=ALU.mult, op1=ALU.add)
            nc.vector.reciprocal(rn, t1)

            # permute q-norm reciprocals into gram layout [32h'+i]
            rnqg = sm.tile([128, 2], F32, tag="rnqg")
            nc.vector.memset(rnqg, 0.0)
            for h in range(HEADS):
                s, j = divmod(h, 4)
                for (p0, col, i0, ln) in _qk_chan_runs(CH * h):
                    nc.sync.dma_start(
                        out=rnqg[32 * j + i0:32 * j + i0 + ln, s:s + 1],
                        in_=rn[p0:p0 + ln, col:col + 1])
            nc.vector.tensor_mul(rnqg, rnqg, tempg_sb)

            # k-norm reciprocals as free-dim rows, broadcast per 32-block
            trr = tr_ps.tile([128, 384], F32, tag="trps")
            for j in range(3):
                nc.tensor.transpose(trr[0:1, 128 * j:128 * (j + 1)],
                                    rn[:, j:j + 1], ident_f32)
            rnrow = sm.tile([1, 384], F32, tag="rnrow")
            nc.vector.tensor_copy(rnrow, trr[0:1, 0:384])
            # partition_broadcast only supports dst partition base 0: broadcast
            # the whole row once, then 32-aligned copies pick per-head slices
            bigbc = sm.tile([128, 384], F32, tag="bigbc")
            nc.gpsimd.partition_broadcast(bigbc, rnrow[0:1, 0:384])
            rnk_bc0 = sm.tile([128, CH], F32, tag="rnkbc0")
            rnk_bc1 = sm.tile([128, CH], F32, tag="rnkbc1")
            rnk_bcs = [rnk_bc0, rnk_bc1]
            for h in range(HEADS):
                s, j = divmod(h, 4)
                for (p0, col, i0, ln) in _qk_chan_runs(C + CH * h):
                    nc.vector.tensor_copy(
                        rnk_bcs[s][32 * j:32 * j + 32, i0:i0 + ln],
                        bigbc[32 * j:32 * j + 32,
                              128 * col + p0:128 * col + p0 + ln])

            for s in range(2):
                gsb = sm.tile([128, CH], F32, tag=f"gsb{s}")
                nc.vector.tensor_copy(gsb, g_tiles[s][:, 0:CH])
                nc.vector.tensor_scalar(out=gsb, in0=gsb, scalar1=rnqg[:, s:s + 1],
                                        scalar2=None, op0=ALU.mult)
                nc.vector.tensor_mul(gsb, gsb, rnk_bcs[s])
                mx = sm.tile([128, 1], F32, tag=f"mx{s}")
                nc.vector.tensor_reduce(mx, gsb, axis=mybir.AxisListType.X, op=ALU.max)
                negm = sm.tile([128, 1], F32, tag=f"negm{s}")
                nc.vector.tensor_scalar_mul(negm, mx, -1.0)
                ex = sm.tile([128, CH], F32, tag=f"ex{s}")
                nc.scalar.activation(ex, gsb, AF.Exp, bias=negm, scale=1.0)
                ssum = sm.tile([128, 1], F32, tag=f"ssum{s}")
                nc.vector.tensor_reduce(ssum, ex, axis=mybir.AxisListType.X, op=ALU.add)
                rs = sm.tile([128, 1], F32, tag=f"rs{s}")
                nc.vector.reciprocal(rs, ssum)
                # pad rows get finite garbage (1/24); zero rows in w3 kill it
                nc.vector.tensor_scalar(out=attn_sbs[s][:, 0:CH], in0=ex,
                                        scalar1=rs, scalar2=None, op0=ALU.mult)

                atp = tr_ps.tile([128, 1024], BF16, tag="trps")
                for j in range(4):
                    nc.tensor.transpose(
                        atp[32 * j:32 * j + 32, 0:32],
                        attn_sbs[s][32 * j:32 * j + 32, 0:32],
                        ident_bf[32 * j:32 * j + 32, 32 * j:32 * j + 32],
                        tile_position=(32 * j, 32 * j))
                nc.vector.tensor_copy(attnT_sbs[s], atp[:, 0:32])

        # =========== phase 2: attn @ v + projection ===========
        p2 = ExitStack()
        with p2:
            av_ps = p2.enter_context(tc.tile_pool(name="avps", bufs=2, space="PSUM"))
            pj_ps = p2.enter_context(tc.tile_pool(name="pjps", bufs=3, space="PSUM"))
            y1_pool = p2.enter_context(tc.tile_pool(name="y1", bufs=2))

            NCH = 512
            for nb in range(HW // NCH):
                px = slice(nb * NCH, (nb + 1) * NCH)
                pxr = (slice(4 * nb, 4 * nb + 4), slice(0, W))  # v_res row view
                y1s = []
                for s in range(2):
                    avp = av_ps.tile([128, NCH], F32, tag="avps")
                    vsrc = v_ress[s][:, 4 * nb:4 * nb + 4, :]
                    for j in range(4):
                        nc.tensor.matmul(
                            avp[32 * j:32 * j + 32, :],
                            attnT_sbs[s][32 * j:32 * j + 32, 0:32],
                            vsrc[32 * j:32 * j + 32].rearrange("p r c -> p (r c)"),
                            start=True, stop=True,
                            tile_position=(32 * j, 32 * j))
                    y1 = y1_pool.tile([128, NCH], BF16, tag="y1")
                    nc.vector.tensor_copy(y1, avp)
                    y1s.append(y1)

                pp0 = pj_ps.tile([128, NCH], F32, tag="pj0")
                nc.tensor.matmul(pp0, w3_sb[:, 0, 0:128], y1s[0], start=True, stop=False)
                nc.tensor.matmul(pp0, w3_sb[:, 1, 0:128], y1s[1], start=False, stop=True)
                pp1 = pj_ps.tile([64, NCH], F32, tag="pj1")
                nc.tensor.matmul(pp1, w3_sb[:, 0, 128:192], y1s[0], start=True, stop=False)
                nc.tensor.matmul(pp1, w3_sb[:, 1, 128:192], y1s[1], start=False, stop=True)

                o0 = y1_pool.tile([128, NCH], F32, tag="o0")
                o1 = y1_pool.tile([64, NCH], F32, tag="o1")
                nc.scalar.activation(o0, pp0, AF.Copy)
                nc.vector.tensor_copy(o1, pp1)
                nc.sync.dma_start(out=y_d[0:128, px], in_=o0)
                nc.sync.dma_start(out=y_d[128:192, px], in_=o1)


_PROG = None


def _get_prog():
    global _PROG
    if _PROG is None:
        _PROG = build_program()
    return _PROG


def kernel(x, w_qkv, w_dw, w_proj, temperature):
    x = np.asarray(x, np.float32)
    b = x.shape[0]
    consts = _prep_host(w_qkv, w_dw, w_proj, temperature)
    nc = _get_prog()

    in_maps = []
    for i in range(b):
        m = {"x": np.ascontiguousarray(x[i].reshape(C, HW))}
        m.update(consts)
        in_maps.append(m)

    import time
    t0 = time.time()
    res = run_bass_kernel_spmd(nc, in_maps, core_ids=list(range(b)))
    if not kernel._warmed:
        # the very first NEFF execution on freshly attached cores has been
        # observed to return corrupted data once; rerun to be safe
        kernel._warmed = True
        res = run_bass_kernel_spmd(nc, in_maps, core_ids=list(range(b)))
    kernel.last_wall_s = time.time() - t0
    kernel.last_exec_ns = res.exec_time_ns
    out = np.stack([res.results[i]["y"].reshape(C, H, W) for i in range(b)])
    return out.astype(np.float32)


kernel._warmed = False



# revision 5
# speedup vs baseline: 1.3804x; 1.3804x over previous
"""DilatedMDTA Trainium2 kernel: batch-parallel over 8 NeuronCores.

Reference computation (per batch image, c=192 channels, 128x128 pixels):
  qkv  = w_qkv @ x                      (1x1 conv, 192 -> 576)
  qkv  = depthwise 3x3 dilation-2 conv  (per-channel, padding 2)
  q,k,v = split(qkv); per head (8 heads, 24 ch):
  q,k L2-normalized over pixels; attn = softmax(q @ k^T * temp); out = attn @ v
  y = w_proj @ out                      (1x1 conv, 192 -> 192)

Layout strategy (per core, one image):
  - channels on SBUF partitions, pixels on the free dim
  - QKV projection: bf16 matmuls, host-permuted weights pad v-heads to
    32-aligned partition blocks (640 output rows)
  - q,k depthwise conv: fp8(e4m3) DoubleRow diagonal matmuls, 2 taps per
    instruction as the two K-tiles (constant free-dim stride between the
    two shifted windows); weights are scaled x8 on host (cancelled in the
    L2 norm, eps scaled to match)
  - v depthwise conv is folded into attn @ v: out = sum_t A_t @ shift_t(vp)
    with A_t = blockdiag(attn^T) row-scaled by the v conv weights, vp kept
    pre-conv in a zero-halo SBUF buffer
  - gram (q@kT): PE-transpose q,k tiles to [pixel, ch] in bf16, cast to fp8
    on the PSUM->SBUF copy, fp8 DoubleRow matmuls over pixel-block pairs
  - norms/softmax: fp32 on DVE/ACT; norm permute via 0/1 matmuls
"""

import os
import sys

sys.path.insert(0, "/opt/trn_rl_repo")

import numpy as np
import ml_dtypes

import concourse.bass as bass
import concourse.mybir as mybir
import concourse.tile as tile
from concourse.bass_utils import run_bass_kernel_spmd
from concourse.masks import make_identity

BF16 = mybir.dt.bfloat16
FP8 = mybir.dt.float8e4
F32 = mybir.dt.float32
AF = mybir.ActivationFunctionType
ALU = mybir.AluOpType
NPBF16 = ml_dtypes.bfloat16
NPFP8 = ml_dtypes.float8_e4m3
PM_DR = mybir.MatmulPerfMode.DoubleRow

C = 192          # input/output channels
C3 = 576         # qkv channels
CP = 640         # padded qkv channels (v heads 32-aligned)
HEADS = 8
CH = 24          # channels per head
H = 128
W = 128
HW = H * W
NB = 8           # row-bands for phase 1
BROWS = 16       # output rows per band
RB = BROWS + 4   # band buffer rows (2-row halo each side)
WP = W + 4       # band buffer cols (2-col halo each side)
EPS = 1e-6
SDW = 8.0        # host scale on q,k depthwise weights (fp8 range)

# taps in (dy, dx); pairs of consecutive taps share one DoubleRow matmul,
# the 10th slot repeats tap 8 with zero weight
TAPS = [(-2, -2), (-2, 0), (-2, 2), (0, -2), (0, 0), (0, 2), (2, -2), (2, 0), (2, 2)]
TAPS10 = TAPS + [TAPS[-1]]
# tap index in the 3x3 kernel (ky*3+kx) for each entry of TAPS
TAP_KIDX = [((dy // 2 + 1) * 3 + (dx // 2 + 1)) for dy, dx in TAPS]
NPAIR = 5
PAIR_DELTA = [
    (TAPS10[2 * p + 1][0] - TAPS10[2 * p][0]) * WP
    + (TAPS10[2 * p + 1][1] - TAPS10[2 * p][1])
    for p in range(NPAIR)
]


def _new_row(o):
    """qkv output channel -> padded row index (v heads 32-aligned)."""
    if o < 384:  # q, k unchanged
        return o
    vc = o - 384
    h, i = divmod(vc, CH)
    return 384 + 32 * h + i


def _prep_host(w_qkv, w_dw, w_proj, temperature):
    """All weight-derived constants, computed on host in numpy."""
    w_qkv = np.asarray(w_qkv, np.float32)
    w_dw = np.asarray(w_dw, np.float32).reshape(C3, 9)
    w_proj = np.asarray(w_proj, np.float32)
    temperature = np.asarray(temperature, np.float32).reshape(HEADS)

    w1p = np.zeros((CP, C), np.float32)
    for o in range(C3):
        w1p[_new_row(o)] = w_qkv[o]
    w1t = np.ascontiguousarray(w1p.T).astype(NPBF16)          # [192, 640]

    # q,k conv: fp8 diag pairs [3 tiles, 5 pairs, 128 k, 2 i, 128 m]
    dw8 = np.clip(w_dw[0:384] * SDW, -240, 240).astype(NPFP8).astype(np.float32)
    dpair = np.zeros((3, NPAIR, 128, 2, 128), np.float32)
    for t in range(3):
        for p in range(NPAIR):
            for i in range(2):
                ti = 2 * p + i
                if ti >= 9:
                    continue  # zero-weight slot
                np.fill_diagonal(dpair[t, p, :, i, :],
                                 dw8[128 * t:128 * (t + 1), TAP_KIDX[ti]])
    dpair = dpair.astype(NPFP8)

    # v conv weights in 32-block layout: [256 rows, 9 taps] f32
    wvcol = np.zeros((256, 9), np.float32)
    for h in range(HEADS):
        for i in range(CH):
            wvcol[32 * h + i] = w_dw[384 + CH * h + i, TAP_KIDX]

    w3p = np.zeros((256, C), np.float32)
    for h in range(HEADS):
        for i in range(CH):
            w3p[32 * h + i] = w_proj[:, CH * h + i]
    w3p = w3p.astype(NPBF16)                                   # [256, 192]

    tempg = np.zeros((256, 1), np.float32)
    for h in range(HEADS):
        tempg[32 * h:32 * h + CH, 0] = temperature[h]

    # q-norm permute matrices (24-block rn layout -> 32-block gram layout)
    # rnqg[32j+i, s] = rn[part(c), col(c)], c = 24*(4s+j)+i
    perm = np.zeros((3, 128, 128), np.float32)
    for s in range(2):
        for j in range(4):
            for i in range(CH):
                c = CH * (4 * s + j) + i
                m = 32 * j + i
                if s == 0:
                    perm[0, c, m] = 1.0            # col 0 -> rnqg col 0
                elif c < 128:
                    perm[1, c, m] = 1.0            # col 0 -> rnqg col 1
                else:
                    perm[2, c - 128, m] = 1.0      # col 1 -> rnqg col 1

    return {"w1t": w1t, "dpair": dpair, "wvcol": wvcol, "w3": w3p,
            "tempg": tempg, "perm": perm}


def _qk_chan_runs(c):
    """qkv channel c (q:0-191, k:192-383) -> (partition, tile-col) runs of 24."""
    runs = []
    start = c
    end = c + CH
    while start < end:
        col = start // 128
        p0 = start % 128
        ln = min(end - start, 128 - p0)
        runs.append((p0, col, start - c, ln))
        start += ln
    return runs


def _pair_view(band, t, r0, dx0, delta):
    """Overlapping DoubleRow rhs: [128, 2(pair, stride delta), 4, W]."""
    v = band[:, t, r0:r0 + 4, 2 + dx0:2 + dx0 + W].unsqueeze(1)
    v = v.broadcast_to([128, 2, 4, W])
    ap = v.ap
    ap[1] = [delta, 2]
    v.ap = ap
    return v


def build_program():
    from concourse import bacc
    nc = bacc.Bacc(None, target_bir_lowering=False)

    x_d = nc.dram_tensor("x", [C, HW], F32, kind="ExternalInput")
    w1t_d = nc.dram_tensor("w1t", [C, CP], BF16, kind="ExternalInput")
    dpair_d = nc.dram_tensor("dpair", [3, NPAIR, 128, 2, 128], FP8,
                             kind="ExternalInput")
    wvcol_d = nc.dram_tensor("wvcol", [256, 9], F32, kind="ExternalInput")
    w3_d = nc.dram_tensor("w3", [256, C], BF16, kind="ExternalInput")
    tempg_d = nc.dram_tensor("tempg", [256, 1], F32, kind="ExternalInput")
    perm_d = nc.dram_tensor("perm", [3, 128, 128], F32, kind="ExternalInput")
    y_d = nc.dram_tensor("y", [C, HW], F32, kind="ExternalOutput")

    with tile.TileContext(nc) as tc:
        _emit(nc, tc, x_d, w1t_d, dpair_d, wvcol_d, w3_d, tempg_d, perm_d, y_d)
    nc.compile()
    return nc


def _emit(nc, tc, x_d, w1t_d, dpair_d, wvcol_d, w3_d, tempg_d, perm_d, y_d):
    from contextlib import ExitStack

    ctx = ExitStack()
    with ctx:
        consts = ctx.enter_context(tc.tile_pool(name="consts", bufs=1))
        persist = ctx.enter_context(tc.tile_pool(name="persist", bufs=1))

        # ---- constants ----
        w1a = consts.tile([128, CP], BF16, tag="w1a")
        w1b = consts.tile([64, CP], BF16, tag="w1b")
        dpair_sb = consts.tile([128, 3, NPAIR, 2, 128], FP8, tag="dpair")
        wvcol_sb = consts.tile([128, 2, 9], F32, tag="wvcol")
        w3_sb = consts.tile([128, 2, C], BF16, tag="w3")
        ident_bf = consts.tile([128, 128], BF16, tag="idbf")
        ident_f32 = consts.tile([128, 128], F32, tag="idf32")
        tempg_sb = consts.tile([128, 2], F32, tag="tempg")
        perm_sb = consts.tile([128, 3, 128], F32, tag="perm")

        nc.sync.dma_start(out=w1a, in_=w1t_d[0:128, :])
        nc.sync.dma_start(out=w1b, in_=w1t_d[128:192, :])
        nc.sync.dma_start(out=dpair_sb,
                          in_=dpair_d[:].rearrange("t p k i m -> k t p i m"))
        nc.sync.dma_start(out=wvcol_sb,
                          in_=wvcol_d[:].rearrange("(s p) t -> p s t", s=2))
        nc.sync.dma_start(out=w3_sb, in_=w3_d[:].rearrange("(k p) m -> p k m", k=2))
        nc.sync.dma_start(out=tempg_sb,
                          in_=tempg_d[:].rearrange("(s p) o -> p (s o)", s=2))
        nc.sync.dma_start(out=perm_sb,
                          in_=perm_d[:].rearrange("n k m -> k n m"))
        make_identity(nc, ident_bf)
        make_identity(nc, ident_f32)

        # ---- persistent SBUF ----
        # vp: pre-conv v in 32-block layout with a 2-pixel zero halo
        vp = persist.tile([128, 2, H + 4, WP], BF16, tag="vp")
        sqacc = persist.tile([128, 3 * NB], F32, tag="sqacc")
        attn_sb0 = persist.tile([128, 32], BF16, tag="attn0")
        attn_sb1 = persist.tile([128, 32], BF16, tag="attn1")
        attn_sbs = [attn_sb0, attn_sb1]
        abd0 = persist.tile([128, 128], BF16, tag="abd0")
        abd1 = persist.tile([128, 128], BF16, tag="abd1")
        abds = [abd0, abd1]
        at0 = persist.tile([128, 9, 128], BF16, tag="at0")
        at1 = persist.tile([128, 9, 128], BF16, tag="at1")
        at_sbs = [at0, at1]

        nc.vector.memset(attn_sb0, 0.0)
        nc.vector.memset(attn_sb1, 0.0)
        nc.vector.memset(abd0, 0.0)
        nc.vector.memset(abd1, 0.0)
        # vp halo zeros (interior is fully written by the proj copies)
        nc.vector.memset(vp[:, :, 0:2, :], 0.0)
        nc.vector.memset(vp[:, :, H + 2:H + 4, :], 0.0)
        nc.vector.memset(vp[:, :, 2:H + 2, 0:2], 0.0)
        nc.vector.memset(vp[:, :, 2:H + 2, WP - 2:WP], 0.0)

        # preload the sqrt ACT table used in phase 1.5 so the serial
        # norm/softmax section doesn't pay the ~2.7us table load
        warm = persist.tile([128, 1], F32, tag="actwarm")
        nc.vector.memset(warm, 1.0)
        nc.scalar.activation(warm, warm, AF.Sqrt)

        # =========== phase 1: qkv + conv + transposes + gram ===========
        p1 = ExitStack()
        with p1:
            xin = p1.enter_context(tc.tile_pool(name="xin", bufs=4))
            xbf_pool = p1.enter_context(tc.tile_pool(name="xbf", bufs=2))
            qkvb_pool = p1.enter_context(tc.tile_pool(name="qkvb", bufs=2))
            qkb_pool = p1.enter_context(tc.tile_pool(name="qkb", bufs=2))
            qt_pool = p1.enter_context(tc.tile_pool(name="qt", bufs=4))
            qkv_ps = p1.enter_context(tc.tile_pool(name="qkvps", bufs=2, space="PSUM"))
            conv_ps = p1.enter_context(tc.tile_pool(name="convps", bufs=2, space="PSUM"))
            tr_ps = p1.enter_context(tc.tile_pool(name="trps", bufs=2, space="PSUM"))
            g_ps = p1.enter_context(tc.tile_pool(name="gps", bufs=1, space="PSUM"))

            # full-bank pitch (512 f32) so partition-offset slices stay
            # zero-region aligned; only cols 0:CH are used
            g0 = g_ps.tile([128, 512], F32, tag="g0")
            g1 = g_ps.tile([128, 512], F32, tag="g1")
            g_tiles = [g0, g1]
            for gt in g_tiles:
                nc.vector.memset(gt[:, 0:CH], 0.0)

            copy_rr = [0]  # round-robin PSUM->SBUF copies between DVE and ACT

            def psum_copy(dst, src):
                if copy_rr[0] % 2 == 0:
                    nc.vector.tensor_copy(dst, src)
                else:
                    nc.scalar.activation(dst, src, AF.Copy)
                copy_rr[0] += 1

            # qkv_band (q,k tiles only, fp8) has 2-col zero pads on each side
            # and zero-padded out-of-image rows so conv taps never clip
            for b in range(NB):
                buf_base = BROWS * b - 2
                lo = max(0, buf_base)
                hi = min(H, BROWS * b + BROWS + 2)
                l0 = lo - buf_base
                l1 = hi - buf_base

                x_bf = xbf_pool.tile([128, 2, RB, W], BF16, tag="xbf")
                qkv_band = qkvb_pool.tile([128, 3, RB, WP], FP8, tag="qkvband")
                qk_band = qkb_pool.tile([128, 3, BROWS, W], BF16, tag="qkband")
                # zero the column pads (rotating slot may hold stale data)
                nc.vector.memset(qkv_band[:, :, :, 0:2], 0.0)
                nc.vector.memset(qkv_band[:, :, :, WP - 2:WP], 0.0)
                if l0 > 0:
                    nc.vector.memset(qkv_band[:, :, 0:l0, :], 0.0)
                if l1 < RB:
                    nc.vector.memset(qkv_band[:, :, l1:RB, :], 0.0)

                # -- x load + cast --
                for t, pt in ((0, 128), (1, 64)):
                    r = l0
                    while r < l1:
                        rr = min(r + 4, l1)
                        n = rr - r
                        xf = xin.tile([128, 4, W], F32, tag="xf32")
                        nc.sync.dma_start(
                            out=xf[:pt, :n, :],
                            in_=x_d[t * 128:t * 128 + pt,
                                    (buf_base + r) * W:(buf_base + rr) * W]
                            .rearrange("p (r c) -> p r c", c=W),
                        )
                        nc.gpsimd.tensor_copy(x_bf[:pt, t, r:rr, :], xf[:pt, :n, :])
                        r = rr

                # -- qkv projection (bf16): q,k tiles -> fp8 band; v -> vp --
                for mt in range(5):
                    mc = slice(mt * 128, (mt + 1) * 128)
                    r = l0
                    while r < l1:
                        rr = min(r + 4, l1)
                        n = rr - r
                        ps = qkv_ps.tile([128, 4, W], F32, tag="qkvps")
                        nc.tensor.matmul(ps[:, :n, :], w1a[:, mc], x_bf[:, 0, r:rr, :],
                                         start=True, stop=False)
                        nc.tensor.matmul(ps[:, :n, :], w1b[:64, mc], x_bf[:64, 1, r:rr, :],
                                         start=False, stop=True)
                        if mt < 3:
                            psum_copy(qkv_band[:, mt, r:rr, 2:2 + W], ps[:, :n, :])
                        else:
                            psum_copy(vp[:, mt - 3, buf_base + r + 2:buf_base + rr + 2,
                                         2:2 + W], ps[:, :n, :])
                        r = rr

                # -- q,k depthwise conv: 5 fp8 DoubleRow pair-matmuls/chunk --
                for t in range(3):
                    for q in range(BROWS // 4):
                        oy0 = BROWS * b + 4 * q
                        cps = conv_ps.tile([128, 4, W], F32, tag="convps")
                        for p in range(NPAIR):
                            dy0, dx0 = TAPS10[2 * p]
                            r0 = oy0 + dy0 - buf_base
                            rhs = _pair_view(qkv_band, t, r0, dx0, PAIR_DELTA[p])
                            nc.tensor.matmul(cps, dpair_sb[:, t, p], rhs,
                                             start=(p == 0), stop=(p == NPAIR - 1),
                                             perf_mode=PM_DR)
                        lq = oy0 - BROWS * b
                        psum_copy(qk_band[:, t, lq:lq + 4, :], cps)

                # -- transpose q,k pixel-blocks (bf16) + fp8 DoubleRow gram --
                for pb2 in range(BROWS // 2):
                    qt = qt_pool.tile([128, 2, 384], BF16, tag="qt")
                    for half in range(2):
                        pb = 2 * pb2 + half
                        trp = tr_ps.tile([128, 384], BF16, tag="trps")
                        for t in range(3):
                            nc.tensor.transpose(trp[:, t * 128:(t + 1) * 128],
                                                qk_band[:, t, pb, :], ident_bf)
                        psum_copy(qt[:, half, :], trp)
                    first = (b == 0 and pb2 == 0)
                    last = (b == NB - 1 and pb2 == BROWS // 2 - 1)
                    for h in range(HEADS):
                        s, j = divmod(h, 4)
                        for half in range(2):
                            nc.tensor.matmul(
                                g_tiles[s][32 * j:32 * j + CH, 0:CH],
                                qt[:, half, CH * h:CH * h + CH],
                                qt[:, half, C + CH * h:C + CH * h + CH],
                                start=(first and half == 0),
                                stop=(last and half == 1),
                                tile_position=(0, 32 * j),
                                skip_group_check=True,
                            )

                # -- squared sums for norms (in-place square of qk_band on ACT) --
                for t in range(3):
                    nc.scalar.activation(
                        qk_band[:, t], qk_band[:, t], AF.Square,
                        accum_out=sqacc[:, (t * NB + b):(t * NB + b + 1)],
                    )

            # =========== phase 1.5: norms + softmax + A_t build ===========
            sm = p1.enter_context(tc.tile_pool(name="smax", bufs=1))
            rqp = tr_ps.tile([128, 2], F32, tag="trps")

            sqsum = sm.tile([128, 3], F32, tag="sqsum")
            nc.vector.tensor_reduce(
                sqsum, sqacc[:].rearrange("p (t b) -> p t b", b=NB),
                axis=mybir.AxisListType.X, op=ALU.add)
            nrm0 = sm.tile([128, 3], F32, tag="nrm0")
            nc.scalar.activation(nrm0, sqsum, AF.Sqrt)
            # one Newton step: nrm = 0.5*(nrm0 + s/nrm0)
            rrec = sm.tile([128, 3], F32, tag="rrec")
            nc.vector.reciprocal(rrec, nrm0)
            t1 = sm.tile([128, 3], F32, tag="t1")
            nc.vector.tensor_mul(t1, sqsum, rrec)
            nc.vector.tensor_add(t1, t1, nrm0)
            rn = sm.tile([128, 3], F32, tag="rn")
            # conv weights were scaled by SDW -> eps scales too
            nc.vector.tensor_scalar(out=t1, in0=t1, scalar1=0.5, scalar2=EPS * SDW,
                                    op0=ALU.mult, op1=ALU.add)
            nc.vector.reciprocal(rn, t1)

            # permute q-norm reciprocals into gram layout [32h'+i] via 0/1 matmuls
            nc.tensor.matmul(rqp[:, 0:1], perm_sb[:, 0, :], rn[:, 0:1],
                             start=True, stop=True)
            nc.tensor.matmul(rqp[:, 1:2], perm_sb[:, 1, :], rn[:, 0:1],
                             start=True, stop=False)
            nc.tensor.matmul(rqp[:, 1:2], perm_sb[0:64, 2, :], rn[0:64, 1:2],
                             start=False, stop=True)
            rnqg = sm.tile([128, 2], F32, tag="rnqg")
            nc.vector.tensor_copy(rnqg, rqp)
            nc.vector.tensor_mul(rnqg, rnqg, tempg_sb)

            # k-norm reciprocals as free-dim rows, broadcast per 32-block
            trr = tr_ps.tile([128, 384], F32, tag="trps")
            for j in range(3):
                nc.tensor.transpose(trr[0:1, 128 * j:128 * (j + 1)],
                                    rn[:, j:j + 1], ident_f32)
            rnrow = sm.tile([1, 384], F32, tag="rnrow")
            nc.vector.tensor_copy(rnrow, trr[0:1, 0:384])
            # partition_broadcast only supports dst partition base 0: broadcast
            # the whole row once, then 32-aligned copies pick per-head slices
            bigbc = sm.tile([128, 384], F32, tag="bigbc")
            nc.gpsimd.partition_broadcast(bigbc, rnrow[0:1, 0:384])
            rnk_bc0 = sm.tile([128, CH], F32, tag="rnkbc0")
            rnk_bc1 = sm.tile([128, CH], F32, tag="rnkbc1")
            rnk_bcs = [rnk_bc0, rnk_bc1]
            for h in range(HEADS):
                s, j = divmod(h, 4)
                for (p0, col, i0, ln) in _qk_chan_runs(C + CH * h):
                    nc.vector.tensor_copy(
                        rnk_bcs[s][32 * j:32 * j + 32, i0:i0 + ln],
                        bigbc[32 * j:32 * j + 32,
                              128 * col + p0:128 * col + p0 + ln])

            for s in range(2):
                gsb = sm.tile([128, CH], F32, tag=f"gsb{s}")
                nc.vector.tensor_copy(gsb, g_tiles[s][:, 0:CH])
                nc.vector.tensor_scalar(out=gsb, in0=gsb, scalar1=rnqg[:, s:s + 1],
                                        scalar2=None, op0=ALU.mult)
                nc.vector.tensor_mul(gsb, gsb, rnk_bcs[s])
                mx = sm.tile([128, 1], F32, tag=f"mx{s}")
                nc.vector.tensor_reduce(mx, gsb, axis=mybir.AxisListType.X, op=ALU.max)
                negm = sm.tile([128, 1], F32, tag=f"negm{s}")
                nc.vector.tensor_scalar_mul(negm, mx, -1.0)
                ex = sm.tile([128, CH], F32, tag=f"ex{s}")
                nc.scalar.activation(ex, gsb, AF.Exp, bias=negm, scale=1.0)
                ssum = sm.tile([128, 1], F32, tag=f"ssum{s}")
                nc.vector.tensor_reduce(ssum, ex, axis=mybir.AxisListType.X, op=ALU.add)
                rs = sm.tile([128, 1], F32, tag=f"rs{s}")
                nc.vector.reciprocal(rs, ssum)
                # pad rows get finite garbage (1/24); the zero pad rows of vp
                # and the zero cols of w3 kill it downstream
                nc.vector.tensor_scalar(out=attn_sbs[s][:, 0:CH], in0=ex,
                                        scalar1=rs, scalar2=None, op0=ALU.mult)

                # transpose attn 32-blocks, assemble block-diagonal attn^T
                atp = tr_ps.tile([128, 1024], BF16, tag="trps")
                for j in range(4):
                    nc.tensor.transpose(
                        atp[32 * j:32 * j + 32, 0:32],
                        attn_sbs[s][32 * j:32 * j + 32, 0:32],
                        ident_bf[32 * j:32 * j + 32, 32 * j:32 * j + 32],
                        tile_position=(32 * j, 32 * j))
                for j in range(4):
                    nc.vector.tensor_copy(
                        abds[s][32 * j:32 * j + 32, 32 * j:32 * j + 32],
                        atp[32 * j:32 * j + 32, 0:32])
                # A_t = blockdiag(attn^T) row-scaled by v conv weight tap t
                for t in range(9):
                    nc.vector.tensor_scalar(
                        out=at_sbs[s][:, t, :], in0=abds[s],
                        scalar1=wvcol_sb[:, s, t:t + 1], scalar2=None,
                        op0=ALU.mult)

        # =========== phase 2: fused (attn o conv) @ vp + projection ===========
        p2 = ExitStack()
        with p2:
            av_ps = p2.enter_context(tc.tile_pool(name="avps", bufs=2, space="PSUM"))
            pj_ps = p2.enter_context(tc.tile_pool(name="pjps", bufs=3, space="PSUM"))
            y1_pool = p2.enter_context(tc.tile_pool(name="y1", bufs=2))

            NCH = 512
            for nb in range(HW // NCH):
                px = slice(nb * NCH, (nb + 1) * NCH)
                y1s = []
                for s in range(2):
                    avp = av_ps.tile([128, NCH], F32, tag="avps")
                    for ti, (dy, dx) in enumerate(TAPS):
                        rhs = vp[:, s, 4 * nb + 2 + dy:4 * nb + 6 + dy,
                                 2 + dx:2 + dx + W]
                        nc.tensor.matmul(avp[:].rearrange("p (r c) -> p r c", c=W),
                                         at_sbs[s][:, ti, :], rhs,
                                         start=(ti == 0), stop=(ti == 8))
                    y1 = y1_pool.tile([128, NCH], BF16, tag="y1")
                    nc.vector.tensor_copy(y1, avp)
                    y1s.append(y1)

                pp0 = pj_ps.tile([128, NCH], F32, tag="pj0")
                nc.tensor.matmul(pp0, w3_sb[:, 0, 0:128], y1s[0], start=True, stop=False)
                nc.tensor.matmul(pp0, w3_sb[:, 1, 0:128], y1s[1], start=False, stop=True)
                pp1 = pj_ps.tile([64, NCH], F32, tag="pj1")
                nc.tensor.matmul(pp1, w3_sb[:, 0, 128:192], y1s[0], start=True, stop=False)
                nc.tensor.matmul(pp1, w3_sb[:, 1, 128:192], y1s[1], start=False, stop=True)

                o0 = y1_pool.tile([128, NCH], F32, tag="o0")
                o1 = y1_pool.tile([64, NCH], F32, tag="o1")
                nc.scalar.activation(o0, pp0, AF.Copy)
                nc.vector.tensor_copy(o1, pp1)
                nc.sync.dma_start(out=y_d[0:128, px], in_=o0)
                nc.sync.dma_start(out=y_d[128:192, px], in_=o1)


_PROG = None


def _get_prog():
    global _PROG
    if _PROG is None:
        _PROG = build_program()
    return _PROG


def kernel(x, w_qkv, w_dw, w_proj, temperature):
    x = np.asarray(x, np.float32)
    b = x.shape[0]
    consts = _prep_host(w_qkv, w_dw, w_proj, temperature)
    nc = _get_prog()

    in_maps = []
    for i in range(b):
        m = {"x": np.ascontiguousarray(x[i].reshape(C, HW))}
        m.update(consts)
        in_maps.append(m)

    import time
    t0 = time.time()
    res = run_bass_kernel_spmd(nc, in_maps, core_ids=list(range(b)))
    if not kernel._warmed:
        # the very first NEFF execution on freshly attached cores has been
        # observed to return corrupted data once; rerun to be safe
        kernel._warmed = True
        res = run_bass_kernel_spmd(nc, in_maps, core_ids=list(range(b)))
    kernel.last_wall_s = time.time() - t0
    kernel.last_exec_ns = res.exec_time_ns
    out = np.stack([res.results[i]["y"].reshape(C, H, W) for i in range(b)])
    return out.astype(np.float32)


kernel._warmed = False
